# revision 37
# baseline (speedup 1.0000x reference)
import sys

if "/opt/trn_rl_repo" not in sys.path:
    sys.path.insert(0, "/opt/trn_rl_repo")

import numpy as np

B, H, W = 4, 512, 512
N_CORES = 8
HS = H // 2
K = 1.0 / 4.5
C0 = 0.040093331769199714
C1 = 0.0007997721694363273

_NC = None

PAIR_ORDER = [
    (0, 1), (0, 2), (1, 0),
    (2, 0), (1, 1), (1, -1),
    (2, 2), (2, -2), (1, 2),
    (2, -1), (1, -2), (2, 1),
]
SLOT = {p: i for i, p in enumerate(PAIR_ORDER)}


def _build_nc():
    import math

    import concourse.bass as bass
    import concourse.mybir as mybir

    dt = mybir.dt
    AF = mybir.ActivationFunctionType
    OP = mybir.AluOpType
    f16 = dt.float16
    f32 = dt.float32
    k = K
    s8 = math.sqrt(8.0)

    nc = bass.Bass(detect_race_conditions=False)

    fA_d = nc.dram_tensor("fA", [128, 6, 516], f16, kind="ExternalInput")
    fB_d = nc.dram_tensor("fB", [128, 6, 516], f16, kind="ExternalInput")
    wuv_d = nc.dram_tensor("wuv", [128, 2048], f16, kind="ExternalInput")
    id_d = nc.dram_tensor("ident", [128, 128], f16, kind="ExternalInput")
    out_d = nc.dram_tensor("out", [128, 1024], f32, kind="ExternalOutput")

    def sb(name, shape, dtype=f16):
        return nc.alloc_sbuf_tensor(name, shape, dtype).ap()

    fA = sb("fA_t", [128, 6, 516])
    fB = sb("fB_t", [128, 6, 516])
    wuv = sb("wuv_t", [128, 2, 1024])
    wu = wuv[:, 0, :]
    wv = wuv[:, 1, :]
    ident = sb("id_t", [128, 128])
    uu = sb("uu", [128, 1024])
    vv = sb("vv", [128, 1024])
    uv = sb("uv", [128, 1024])
    r2 = sb("r2", [128, 1024])
    lnr = sb("lnr", [128, 1024], f32)
    ir2 = sb("ir2", [128, 1024])
    ss = sb("ss", [128, 1024])
    cs = sb("cs", [128, 1024])
    cs43 = sb("cs43", [128, 1024])
    m12 = sb("m12", [128, 1024])
    m1m2 = sb("m1m2", [128, 1024])
    q = sb("q", [128, 1024])
    ser = sb("ser", [128, 1024])
    w_all = sb("w_all", [128, 12, 1024])
    pst = sb("pst", [128, 12, 1024])
    prod = sb("prod", [128, 12, 1024])
    tfull = sb("tfull", [128, 1024])
    ofull = sb("ofull", [128, 1024])
    cen03 = sb("cen03", [128, 1024])
    outt = sb("outt", [128, 1024], f32)
    dummy = sb("dummy_t", [128, 1], f32)
    dummy_in = sb("dummy_in", [128, 1], f32)

    acc = nc.alloc_psum_tensor("acc", [128, 1024], f32).ap()

    espec = {
        (0, 1): (ss, -k, 0.0),
        (0, 2): (ss, -4 * k, 0.0),
        (1, 0): (ss, k, -k),
        (2, 0): (ss, 4 * k, -4 * k),
        (1, 1): (cs, -2 * k, -k),
        (1, -1): (cs, 2 * k, -k),
        (2, 2): (cs, -8 * k, -4 * k),
        (2, -2): (cs, 8 * k, -4 * k),
        (1, 2): (m12, -3 * k, -k),
        (2, -1): (m12, 3 * k, -4 * k),
        (1, -2): (m1m2, -3 * k, -k),
        (2, 1): (m1m2, 3 * k, -4 * k),
    }

    def V(dx, dy, half=None):
        if dy % 2 == 0:
            t, c0 = fA, 2 + dy
        else:
            t, c0 = fB, 1 + dy
        if half is None:
            return t[:, 2 + dx : 4 + dx, c0 : c0 + 512]
        return t[:, 2 + dx + half, c0 : c0 + 512]

    bias_vals = sorted({0.0, 1e-4, -k, -4 * k, -s8 / 2, C0 - C1})

    with (
        nc.semaphore("squ") as SQU,
        nc.semaphore("sfa") as FA,
        nc.semaphore("sa") as A,
        nc.semaphore("sv") as Vs,
        nc.semaphore("sp") as P,
        nc.semaphore("sy") as SY,
        nc.semaphore("sb") as SB,
    ):
        for bi_i, val in enumerate(bias_vals):
            if (f32, val) in nc.const_aps.aps:
                continue
            t = nc.alloc_sbuf_tensor(f"constb{bi_i}", [128, 1], f32)
            nc.gpsimd.memset(t.ap(), val)
            nc.const_aps.aps[(f32, val)] = t.ap()
        nc.gpsimd.memset(dummy_in, 0.0).then_inc(SB, 1)
        nc.gpsimd.dma_start(ident, id_d[:, :]).then_inc(FA, 16)

        nc.sync.dma_start(wuv[0:64], wuv_d[0:64, :]).then_inc(SQU, 16)
        nc.scalar.dma_start(wuv[64:128], wuv_d[64:128, :]).then_inc(SQU, 16)

        with nc.Block() as block:

            @block.gpsimd
            def _(gpsimd):
                gpsimd.dma_start(fA[:, :, :], fA_d[:, :, :]).then_inc(FA, 16)
                gpsimd.dma_start(fB[:, :, :], fB_d[:, :, :]).then_inc(FA, 16)

            @block.sync
            def _(sync):
                sync.wait_ge(Vs, 11)
                sync.dma_start(out_d[:, 0:512], outt[:, 0:512]).then_inc(SY, 16)
                sync.wait_ge(Vs, 12)
                sync.dma_start(out_d[:, 512:1024], outt[:, 512:1024]).then_inc(SY, 16)

            @block.scalar
            def _(scalar):
                a_count = [0]

                def aop(emit):
                    emit().then_inc(A, 1)
                    a_count[0] += 1

                scalar.wait_ge(SB, 1)
                aop(lambda: scalar.activation(dummy, dummy_in, AF.Exp))
                scalar.wait_ge(SQU, 32)
                aop(lambda: scalar.activation(uu, wu, AF.Square))
                scalar.wait_ge(Vs, 1)
                aop(lambda: scalar.activation(lnr, r2, AF.Ln, bias=1e-4))
                scalar.wait_ge(A, 3)
                aop(lambda: scalar.activation(ir2, lnr, AF.Exp, scale=-1.0))

                def exp_of(p):
                    src_, sc, bi = espec[p]
                    aop(lambda: scalar.activation(
                        w_all[:, SLOT[p], :], src_, AF.Exp, bias=bi, scale=sc))

                scalar.wait_ge(Vs, 2)
                exp_of((0, 1))
                exp_of((0, 2))
                exp_of((1, 0))
                exp_of((2, 0))
                scalar.wait_ge(Vs, 3)
                exp_of((1, 1))
                exp_of((1, -1))
                exp_of((2, 2))
                exp_of((2, -2))
                scalar.wait_ge(Vs, 4)
                exp_of((1, 2))
                exp_of((2, -1))
                scalar.wait_ge(Vs, 5)
                exp_of((1, -2))
                exp_of((2, 1))
                aop(lambda: scalar.activation(q, ss, AF.Square,
                                              bias=-s8 / 2, scale=s8))
                scalar.wait_ge(A, 17)
                aop(lambda: scalar.activation(ser, q, AF.Identity,
                                              bias=C0 - C1, scale=C1))
                scalar.wait_ge(FA, 32)
                aop(lambda: scalar.activation(
                    cen03.rearrange("p (a b) -> p a b", a=2), V(0, 0),
                    AF.Identity, bias=0.0, scale=0.3))
                assert a_count[0] == 19

            @block.vector
            def _(vector):
                def psum_of(p):
                    vector.tensor_tensor(
                        pst[:, SLOT[p], :].rearrange("p (a b) -> p a b", a=2),
                        V(*p), V(-p[0], -p[1]), OP.add)

                def mac_of(lo, hi, a_need):
                    vector.wait_ge(A, a_need)
                    sl = slice(lo, hi)
                    vector.tensor_tensor(
                        prod[:, sl, :], w_all[:, sl, :], pst[:, sl, :], OP.mult
                    ).then_inc(Vs, 1)

                vector.wait_ge(SQU, 32)
                vector.tensor_tensor(vv, wv, wv, OP.mult)
                vector.wait_ge(A, 2)
                vector.tensor_tensor(r2, uu, vv, OP.add).then_inc(Vs, 1)
                vector.tensor_tensor(uv, wu, wv, OP.mult)
                vector.wait_ge(FA, 32)
                psum_of((0, 2))
                psum_of((1, 0))
                vector.wait_ge(A, 4)
                vector.tensor_tensor(ss, vv, ir2, OP.mult).then_inc(Vs, 1)
                vector.tensor_tensor(cs, uv, ir2, OP.mult).then_inc(Vs, 1)
                vector.tensor_scalar_mul(cs43, cs, 4.0 / 3.0)
                vector.tensor_tensor(m12, ss, cs43, OP.add).then_inc(Vs, 1)
                vector.tensor_tensor(m1m2, ss, cs43, OP.subtract).then_inc(Vs, 1)
                vector.wait_ge(FA, 48)
                psum_of((0, 1))
                psum_of((2, 0))
                mac_of(0, 3, 7)
                psum_of((1, 1))
                psum_of((1, -1))
                psum_of((2, 2))
                mac_of(3, 6, 10)
                psum_of((2, -2))
                psum_of((1, 2))
                psum_of((2, -1))
                mac_of(6, 9, 13)
                psum_of((1, -2))
                psum_of((2, 1))
                mac_of(9, 11, 15)
                mac_of(11, 12, 16)
                vector.wait_ge(A, 19)
                vector.wait_ge(P, 2)
                vector.tensor_tensor(tfull, acc, ser, OP.mult)
                vector.tensor_tensor(ofull, cen03, tfull, OP.add)
                vector.tensor_scalar(out=outt[:, 0:512], in0=ofull[:, 0:512],
                                     scalar1=0.0, scalar2=1.0, op0=OP.max,
                                     op1=OP.min).then_inc(Vs, 1)
                vector.tensor_scalar(out=outt[:, 512:1024], in0=ofull[:, 512:1024],
                                     scalar1=0.0, scalar2=1.0, op0=OP.max,
                                     op1=OP.min).then_inc(Vs, 1)

            @block.tensor
            def _(tensor):
                def mm(h, rhs, start, stop=False):
                    return tensor.matmul(out=acc[:, 512 * h : 512 * h + 512],
                                         lhsT=ident, rhs=rhs,
                                         start=start, stop=stop,
                                         skip_group_check=True)

                tensor.wait_ge(FA, 32)
                mm(0, V(0, 0, 0), True)
                mm(1, V(0, 0, 1), True)
                for g, (lo, hi) in enumerate(((0, 3), (3, 6), (6, 9), (9, 11))):
                    tensor.wait_ge(Vs, 6 + g)
                    for i in range(lo, hi):
                        mm(0, prod[:, i, 0:512], False)
                        mm(1, prod[:, i, 512:1024], False)
                tensor.wait_ge(Vs, 10)
                mm(0, prod[:, 11, 0:512], False, True).then_inc(P, 1)
                mm(1, prod[:, 11, 512:1024], False, True).then_inc(P, 1)

    return nc


def _get_nc():
    global _NC
    if _NC is None:
        _NC = _build_nc()
    return _NC


def _make_in_maps(fire_map, wind_u, wind_v):
    from numpy.lib.stride_tricks import sliding_window_view

    ident = np.eye(128, dtype=np.float16)
    in_maps = []
    for b in range(B):
        fp = np.pad(np.asarray(fire_map[b, 0], np.float32), ((2, 2), (2, 3)))
        fp16 = fp.astype(np.float16)
        for t in range(2):
            shard = fp16[t * HS : t * HS + HS + 4]
            swv = sliding_window_view(shard, (6, 516))
            fA = np.ascontiguousarray(swv[::2, 0])
            fB = np.ascontiguousarray(swv[::2, 1])
            wus = (np.asarray(wind_u[b, 0, t * HS : (t + 1) * HS], np.float32)
                   .reshape(128, 1024).astype(np.float16))
            wvs = (np.asarray(wind_v[b, 0, t * HS : (t + 1) * HS], np.float32)
                   .reshape(128, 1024).astype(np.float16))
            wuv = np.ascontiguousarray(np.concatenate([wus, wvs], axis=1))
            in_maps.append(
                {"fA": fA, "fB": fB, "wuv": wuv, "ident": ident})
    return in_maps


def _gather(results):
    out = np.empty((B, 1, H, W), np.float32)
    for ci, r in enumerate(results):
        b, t = divmod(ci, 2)
        out[b, 0, t * HS : (t + 1) * HS] = r["out"].reshape(HS, W)
    return out


def _run(fire_map, wind_u, wind_v, trace=False):
    from concourse.bass_utils import run_bass_kernel_spmd

    in_maps = _make_in_maps(fire_map, wind_u, wind_v)
    res = run_bass_kernel_spmd(_get_nc(), in_maps, list(range(N_CORES)), trace=trace)
    return _gather(res.results), res


def kernel(fire_map, wind_u, wind_v):
    out, _ = _run(fire_map, wind_u, wind_v, trace=False)
    return out
